# revision 9
# baseline (speedup 1.0000x reference)
"""Trainium2 Bass kernel for nn_APSDG_30124900614677 (gnn_message_passing).

Distributed over 8 NeuronCores: nodes sharded contiguously (6250/core, padded
to 6272=49*128). Per GNN layer: node-local transforms (e: GraphConv-norm,
b: Poincare logmap, s: sphere l2norm) -> bf16 all-gather of transformed
features -> per-core edge phase: dma_gather of src rows + segment-sum via
selection-matrix matmuls into PSUM (edges pre-sorted by dst on host)
-> epilogue nonlinearities. History attention + GRU cell are node-local with
host-folded weight products (k/v projections folded into a single matmul
each via softmax/attention algebra).

Self-contained: hardcodes all shapes; host side only uses numpy (+ml_dtypes)
and the concourse Bass stack from /opt/trn_rl_repo.
"""
import os
import sys
import time

sys.path.insert(0, "/opt/trn_rl_repo")

import numpy as np

from concourse import bass, bacc, mybir, tile
from concourse.bass_utils import run_bass_kernel_spmd

try:
    import ml_dtypes
    BF16 = np.dtype(ml_dtypes.bfloat16)
except Exception:  # pragma: no cover
    import jax.numpy as jnp
    BF16 = np.dtype(jnp.bfloat16)

F32 = mybir.dt.float32
BF = mybir.dt.bfloat16
I16 = mybir.dt.int16
AF = mybir.ActivationFunctionType
OP = mybir.AluOpType

# problem constants
N = 50000
E = 800000
D = 384
SUB = 128
W = 5
L = 2
NCORES = 8
OWN = N // NCORES            # 6250
T = 49                       # node tiles per core
NP = T * 128                 # 6272 padded nodes per core
NPAD = NCORES * NP           # 50176
SPLIT = 32768                # int16 gather-table split row
LEAKY = 0.2
EPS = 1e-5
SEPS = 1e-12

_RUN_STATE = {}


# ---------------------------------------------------------------- host prep

def _pad_rows(a, rows):
    out = np.zeros((rows,) + a.shape[1:], a.dtype)
    out[: a.shape[0]] = a
    return out


def _prep(inputs):
    src = np.asarray(inputs["src"]).astype(np.int64)
    dst = np.asarray(inputs["dst"]).astype(np.int64)
    node_emb = np.asarray(inputs["node_emb"], dtype=np.float32)
    history = np.asarray(inputs["history"], dtype=np.float32)

    deg_out = np.maximum(np.bincount(src, minlength=N), 1.0).astype(np.float32)
    deg_in = np.maximum(np.bincount(dst, minlength=N), 1.0).astype(np.float32)
    isqo = deg_out ** -0.5
    isqi = deg_in ** -0.5
    iin = (1.0 / deg_in).astype(np.float32)

    c_abs = float(np.abs(np.asarray(inputs["curv_b"]).reshape(-1)[0]))

    src_pad = (src // OWN) * NP + (src % OWN)

    # ---- per-core edge layout (class A: src_pad < SPLIT, B: >= SPLIT) ----
    core_of = dst // OWN
    percore = []
    cnt = np.zeros((NCORES, T, 2), np.int64)
    for c in range(NCORES):
        m = core_of == c
        sp = src_pad[m]
        dl = dst[m] - c * OWN
        g = dl // 128
        cls = (sp >= SPLIT).astype(np.int64)
        order = np.lexsort((dl, cls, g))
        sp, dl, g, cls = sp[order], dl[order], g[order], cls[order]
        np.add.at(cnt[c], (g, cls), 1)
        percore.append((sp, dl, g, cls))

    KA = np.maximum(1, -(-cnt[:, :, 0].max(0) // 128)).astype(np.int64)
    KB = np.maximum(1, -(-cnt[:, :, 1].max(0) // 128)).astype(np.int64)
    KG = KA + KB
    TILES_TOT = int(KG.sum())
    IDXCOLS = int(8 * KG.sum())

    tile_base = np.zeros((T, 2), np.int64)   # tile index base per (g, cls)
    col_base = np.zeros((T, 2), np.int64)    # idx column base per (g, cls)
    tb = 0
    cb = 0
    for g in range(T):
        tile_base[g, 0] = tb
        tile_base[g, 1] = tb + KA[g]
        col_base[g, 0] = cb
        col_base[g, 1] = cb + 8 * KA[g]
        tb += KG[g]
        cb += 8 * KG[g]

    # ---- fold attention/GRU weights on host (f64 accumulate) ----
    f64 = np.float64
    q_W = np.asarray(inputs["q_W"], f64)
    q_b = np.asarray(inputs["q_b"], f64)
    in_w = np.asarray(inputs["in_w"], f64)
    in_b = np.asarray(inputs["in_b"], f64)
    out_w = np.asarray(inputs["out_w"], f64)
    out_b = np.asarray(inputs["out_b"], f64)
    gru_wih = np.asarray(inputs["gru_wih"], f64)
    gru_bih = np.asarray(inputs["gru_bih"], f64)
    gru_whh = np.asarray(inputs["gru_whh"], f64)
    gru_bhh = np.asarray(inputs["gru_bhh"], f64)

    Wq = q_W @ in_w[:, :D]
    aq = q_b @ in_w[:, :D] + in_b[:D]
    Wk = in_w[:, D:2 * D]
    Wqk = Wq @ Wk.T
    bqk = aq @ Wk.T
    Wv = in_w[:, 2 * D:]
    bv = in_b[2 * D:]
    Wvo = Wv @ out_w
    bvo = bv @ out_w + out_b
    Wvog = Wvo @ gru_whh
    bvog = bvo @ gru_whh + gru_bhh
    Wih = gru_wih
    bih = gru_bih

    def kmaj(wmat, n):
        # [384, n] -> [128, 3, n] with K (row) on partitions
        return np.ascontiguousarray(
            wmat.reshape(3, 128, n).transpose(1, 0, 2)).astype(BF16)

    wqk_h = kmaj(Wqk, D)
    wvo_h = kmaj(Wvo, D)
    wvog_h = kmaj(Wvog, 3 * D)
    wih_h = kmaj(Wih, 3 * D)

    battn = np.zeros((4, 3 * D), np.float64)
    battn[0, :D] = bqk
    battn[1, :D] = bvo
    battn[2] = bvog
    battn[3] = bih
    battn_h = battn.astype(BF16)
    battn_nz = [bool(np.abs(battn[i]).max() > 0) for i in range(4)]

    # layer sub-space weights, K on partitions: [128, 6, 128]
    e_W = np.asarray(inputs["e_W"], np.float32)
    b_W = np.asarray(inputs["b_W"], np.float32)
    s_W = np.asarray(inputs["s_W"], np.float32)
    wsub_h = np.zeros((128, 2 * 3, 128), BF16)
    for l in range(L):
        wsub_h[:, l * 3 + 0] = e_W[l].astype(BF16)
        wsub_h[:, l * 3 + 1] = b_W[l].astype(BF16)
        wsub_h[:, l * 3 + 2] = s_W[l].astype(BF16)

    e_b = np.asarray(inputs["e_b"], np.float32)
    b_bias = np.asarray(inputs["b_bias"], np.float32)
    s_bias = np.asarray(inputs["s_bias"], np.float32)
    brep_h = np.zeros((128, 6, 128), np.float32)
    brep_nz = np.zeros((L, 3), bool)
    for l in range(L):
        for i, b in enumerate((e_b[l], b_bias[l], s_bias[l])):
            brep_h[:, l * 3 + i] = b[None, :]
            brep_nz[l, i] = bool(np.abs(b).max() > 0)

    iota_h = np.tile(np.arange(128, dtype=np.float32)[None, :], (128, 1))
    idf_h = np.eye(128, dtype=np.float32)

    meta = dict(KA=KA, KB=KB, KG=KG, TILES_TOT=TILES_TOT, IDXCOLS=IDXCOLS,
                tile_base=tile_base, col_base=col_base, c_abs=c_abs,
                battn_nz=battn_nz, brep_nz=brep_nz)

    # ---- per-core input maps ----
    in_maps = []
    for c in range(NCORES):
        sp, dl, g, cls = percore[c]
        idx16 = np.zeros((16, IDXCOLS), np.int16)
        drel = np.full((128, TILES_TOT), -1.0, np.float32)

        # rank within (g, cls) block
        blockkey = g * 2 + cls
        # edges are lexsorted by (g, cls, dl): rank = position - block start
        starts = np.zeros(T * 2, np.int64)
        bc = np.bincount(blockkey, minlength=T * 2)
        starts[1:] = np.cumsum(bc)[:-1]
        rank = np.arange(len(sp)) - starts[blockkey]

        val = np.where(cls == 0, sp, sp - SPLIT).astype(np.int16)
        col = col_base[g, cls] + rank // 16
        row = rank % 16
        idx16[row, col] = val

        tt = tile_base[g, cls] + rank // 128
        p = rank % 128
        drel[p, tt] = (dl - g * 128).astype(np.float32)

        idx_h = np.tile(idx16, (8, 1))

        lo = c * OWN
        hi = lo + OWN
        scales = np.ones((128, 3 * T), np.float32)
        for arr, ofs in ((isqo, 0), (isqi, T), (iin, 2 * T)):
            own = _pad_rows(arr[lo:hi, None], NP)[:, 0]
            scales[:, ofs:ofs + T] = own.reshape(T, 128).T

        feat0 = _pad_rows(node_emb[lo:hi], NP)
        hist = np.zeros((NP, W * D), np.float32)
        hist[:OWN] = np.ascontiguousarray(
            history[:, lo:hi, :].transpose(1, 0, 2)).reshape(OWN, W * D)

        in_maps.append({
            "feat0": feat0,
            "hist": hist,
            "idx": idx_h,
            "drel": drel,
            "scales": scales,
            "wsub": wsub_h.reshape(128, 6 * 128),
            "brep": brep_h.reshape(128, 6 * 128),
            "wqk": wqk_h.reshape(128, 3 * D),
            "wvo": wvo_h.reshape(128, 3 * D),
            "wvog": wvog_h.reshape(128, 9 * D),
            "wih": wih_h.reshape(128, 9 * D),
            "battn": battn_h,
            "iota": iota_h,
            "idf": idf_h,
        })
    return in_maps, meta


# ------------------------------------------------------------- device build

def _build(meta):
    KA, KB, KG = meta["KA"], meta["KB"], meta["KG"]
    TILES_TOT, IDXCOLS = meta["TILES_TOT"], meta["IDXCOLS"]
    tile_base, col_base = meta["tile_base"], meta["col_base"]
    c_abs = meta["c_abs"]
    battn_nz = meta["battn_nz"]
    brep_nz = meta["brep_nz"]
    ID_SCALE = float(D) ** -0.5
    phases = os.environ.get("GNN_PHASES", "taex")  # t=transform a=allgather e=edge x=attn

    nc = bacc.Bacc("TRN2", target_bir_lowering=False, debug=False,
                   num_devices=NCORES)

    feat0_d = nc.dram_tensor("feat0", [NP, D], F32, kind="ExternalInput")
    hist_d = nc.dram_tensor("hist", [NP, W * D], F32, kind="ExternalInput")
    idx_d = nc.dram_tensor("idx", [128, IDXCOLS], I16, kind="ExternalInput")
    drel_d = nc.dram_tensor("drel", [128, TILES_TOT], F32, kind="ExternalInput")
    scales_d = nc.dram_tensor("scales", [128, 3 * T], F32, kind="ExternalInput")
    wsub_d = nc.dram_tensor("wsub", [128, 6 * 128], BF, kind="ExternalInput")
    brep_d = nc.dram_tensor("brep", [128, 6 * 128], F32, kind="ExternalInput")
    wqk_d = nc.dram_tensor("wqk", [128, 3 * D], BF, kind="ExternalInput")
    wvo_d = nc.dram_tensor("wvo", [128, 3 * D], BF, kind="ExternalInput")
    wvog_d = nc.dram_tensor("wvog", [128, 9 * D], BF, kind="ExternalInput")
    wih_d = nc.dram_tensor("wih", [128, 9 * D], BF, kind="ExternalInput")
    battn_d = nc.dram_tensor("battn", [4, 3 * D], BF, kind="ExternalInput")
    iota_d = nc.dram_tensor("iota", [128, 128], F32, kind="ExternalInput")
    idf_d = nc.dram_tensor("idf", [128, 128], F32, kind="ExternalInput")
    out_d = nc.dram_tensor("out", [NP, D], F32, kind="ExternalOutput")

    t_own = nc.dram_tensor("t_own", [NP, D], BF, kind="Internal")
    t_full = nc.dram_tensor("t_full", [NPAD, D], BF, kind="Internal",
                            addr_space="Shared")
    featA = nc.dram_tensor("featA", [NP, D], F32, kind="Internal")
    featB = nc.dram_tensor("featB", [NP, D], F32, kind="Internal")

    with tile.TileContext(nc) as tc:
        with tc.tile_pool(name="const", bufs=1) as cpool:
            idx_sb = cpool.tile([128, IDXCOLS], I16)
            drel_sb = cpool.tile([128, TILES_TOT], F32)
            scales_sb = cpool.tile([128, 3 * T], F32)
            wsub_sb = cpool.tile([128, 6 * 128], BF)
            brep_sb = cpool.tile([128, 6 * 128], F32)
            wqk_sb = cpool.tile([128, 3 * D], BF)
            wvo_sb = cpool.tile([128, 3 * D], BF)
            wvog_sb = cpool.tile([128, 9 * D], BF)
            wih_sb = cpool.tile([128, 9 * D], BF)
            battn_sb = cpool.tile([4, 3 * D], BF)
            iota_sb = cpool.tile([128, 128], F32)
            idf_sb = cpool.tile([128, 128], F32)
            ones_sb = cpool.tile([1, 128], BF)
            for sb, dr in ((idx_sb, idx_d), (drel_sb, drel_d),
                           (scales_sb, scales_d), (wsub_sb, wsub_d),
                           (brep_sb, brep_d), (wqk_sb, wqk_d),
                           (wvo_sb, wvo_d), (wvog_sb, wvog_d),
                           (wih_sb, wih_d), (battn_sb, battn_d),
                           (iota_sb, iota_d), (idf_sb, idf_d)):
                nc.sync.dma_start(sb[:], dr[:])
            nc.gpsimd.memset(ones_sb[:], 1.0)

            V = nc.vector
            S_ = nc.scalar

            def norm_scale_chain(pool, ss, kind):
                """[128,1] chains. kind: 'log' artanh(min(sn,1-eps))/max(sn,eps),
                'exp' tanh(sn)/max(sn,eps), 'l2' 1/max(sqrt(ss),1e-12).
                Returns [128,1] f32 scale tile."""
                sn = pool.tile([128, 1], F32, tag="c_sn")
                S_.activation(sn[:], ss[:], AF.Sqrt, scale=c_abs if kind != "l2" else 1.0)
                if kind == "l2":
                    m = pool.tile([128, 1], F32, tag="c_m")
                    V.tensor_scalar_max(m[:], sn[:], SEPS)
                    V.reciprocal(m[:], m[:])
                    return m
                m = pool.tile([128, 1], F32, tag="c_m")
                V.tensor_scalar_max(m[:], sn[:], EPS)
                V.reciprocal(m[:], m[:])
                if kind == "exp":
                    th = pool.tile([128, 1], F32, tag="c_th")
                    S_.activation(th[:], sn[:], AF.Tanh)
                    sc = pool.tile([128, 1], F32, tag="c_sc")
                    V.tensor_tensor(out=sc[:], in0=th[:], in1=m[:], op=OP.mult)
                    return sc
                # log: artanh via 0.5*ln((1+x)/(1-x))
                x = pool.tile([128, 1], F32, tag="c_x")
                V.tensor_scalar_min(x[:], sn[:], 1.0 - EPS)
                a1 = pool.tile([128, 1], F32, tag="c_a1")
                V.tensor_scalar_add(a1[:], x[:], 1.0)
                a2 = pool.tile([128, 1], F32, tag="c_a2")
                V.tensor_scalar(a2[:], x[:], -1.0, 1.0, op0=OP.mult, op1=OP.add)
                V.reciprocal(a2[:], a2[:])
                y = pool.tile([128, 1], F32, tag="c_y")
                V.tensor_tensor(out=y[:], in0=a1[:], in1=a2[:], op=OP.mult)
                ln = pool.tile([128, 1], F32, tag="c_ln")
                S_.activation(ln[:], y[:], AF.Ln)
                sc = pool.tile([128, 1], F32, tag="c_sc")
                V.scalar_tensor_tensor(out=sc[:], in0=ln[:], scalar=0.5,
                                       in1=m[:], op0=OP.mult, op1=OP.mult)
                return sc

            # ================= layer loop =================
            for l in range(L):
                feat_src = feat0_d if l == 0 else featA
                feat_dst = featA if l == 0 else featB

                # ---------- transform ----------
                with (
                    tc.tile_pool(name=f"tf{l}", bufs=3) as pool,
                    tc.tile_pool(name=f"tfp{l}", bufs=2, space="PSUM") as pp,
                ):
                    for t in range(T):
                        ft = pool.tile([128, D], F32, tag="ft")
                        nc.sync.dma_start(ft[:], feat_src[t * 128:(t + 1) * 128, :])
                        tsb = pool.tile([128, D], BF, tag="tsb")
                        junk = pool.tile([128, 128], BF, tag="junk")

                        for i, kind in enumerate(("e", "b", "s")):
                            sl = slice(i * 128, (i + 1) * 128)
                            if kind == "e":
                                xin = ft[:, sl]
                            else:
                                ss = pool.tile([128, 1], F32, tag="c_ss")
                                S_.activation(junk[:], ft[:, sl], AF.Square,
                                              accum_out=ss[:])
                                sc = norm_scale_chain(
                                    pool, ss, "log" if kind == "b" else "l2")
                                xs = pool.tile([128, 128], F32, tag="xs")
                                V.tensor_scalar_mul(xs[:], ft[:, sl], sc[:, 0:1])
                                xin = xs[:]
                            pt = pp.tile([128, 128], F32, tag="tp")
                            nc.tensor.transpose(pt[:], xin, idf_sb[:])
                            xT = pool.tile([128, 128], BF, tag="xT")
                            V.tensor_copy(xT[:], pt[:])
                            pm = pp.tile([128, 128], F32, tag="mm")
                            nc.tensor.matmul(
                                pm[:], lhsT=xT[:],
                                rhs=wsub_sb[:, (l * 3 + i) * 128:(l * 3 + i + 1) * 128],
                                start=True, stop=True)
                            if kind == "e":
                                V.tensor_scalar_mul(tsb[:, sl], pm[:],
                                                    scales_sb[:, t:t + 1])
                            elif brep_nz[l][i]:
                                V.scalar_tensor_tensor(
                                    out=tsb[:, sl], in0=pm[:], scalar=1.0,
                                    in1=brep_sb[:, (l * 3 + i) * 128:(l * 3 + i + 1) * 128],
                                    op0=OP.mult, op1=OP.add)
                            else:
                                V.tensor_copy(tsb[:, sl], pm[:])
                        nc.sync.dma_start(t_own[t * 128:(t + 1) * 128, :], tsb[:])

                # ---------- all-gather ----------
                if "a" in phases:
                    nc.gpsimd.collective_compute(
                        "AllGather", OP.bypass,
                        replica_groups=[list(range(NCORES))],
                        ins=[t_own[:].opt()],
                        outs=[t_full[:].opt()],
                    )

                # ---------- edge phase ----------
                if "e" not in phases:
                    continue
                KGmax = int(KG.max())
                with (
                    tc.tile_pool(name=f"ed{l}", bufs=2) as pool,
                    tc.tile_pool(name=f"edp{l}", bufs=2, space="PSUM") as pp,
                ):
                    KC = int(os.environ.get("GNN_GATHER_CHUNK", "4"))
                    for g in range(T):
                        ka, kb, kg = int(KA[g]), int(KB[g]), int(KG[g])
                        ca, cb = int(col_base[g, 0]), int(col_base[g, 1])
                        tb = int(tile_base[g, 0])
                        msg = pool.tile([128, KGmax, D], BF, tag="msg")

                        def gather(table_ap, kcnt, colofs, chunk0):
                            for c0 in range(0, kcnt, KC):
                                kc = min(KC, kcnt - c0)
                                nc.gpsimd.dma_gather(
                                    out_ap=msg[:, chunk0 + c0:chunk0 + c0 + kc, :],
                                    in_ap=table_ap,
                                    idxs_ap=idx_sb[:, colofs + 8 * c0:
                                                   colofs + 8 * (c0 + kc)],
                                    num_idxs=128 * kc, num_idxs_reg=128 * kc,
                                    elem_size=D)

                        gather(t_full[0:SPLIT, :], ka, ca, 0)
                        gather(t_full[SPLIT:NPAD, :], kb, cb, ka)
                        ps = pp.tile([128, D], F32, tag="eps")
                        for k in range(kg):
                            Smat = pool.tile([128, 128], BF, tag="S")
                            V.tensor_tensor(
                                out=Smat[:],
                                in0=drel_sb[:, tb + k:tb + k + 1].to_broadcast([128, 128]),
                                in1=iota_sb[:], op=OP.is_equal)
                            nc.tensor.matmul(ps[:], lhsT=Smat[:],
                                             rhs=msg[:, k, :],
                                             start=(k == 0), stop=(k == kg - 1))
                        nf = pool.tile([128, D], F32, tag="nf")
                        junk = pool.tile([128, 128], BF, tag="junk")
                        # e: *isqi (+e_b) then leaky
                        et = pool.tile([128, 128], F32, tag="et")
                        if brep_nz[l][0]:
                            V.scalar_tensor_tensor(
                                out=et[:], in0=ps[:, 0:128],
                                scalar=scales_sb[:, T + g:T + g + 1],
                                in1=brep_sb[:, (l * 3) * 128:(l * 3 + 1) * 128],
                                op0=OP.mult, op1=OP.add)
                        else:
                            V.tensor_scalar_mul(et[:], ps[:, 0:128],
                                                scales_sb[:, T + g:T + g + 1])
                        V.scalar_tensor_tensor(out=nf[:, 0:128], in0=et[:],
                                               scalar=LEAKY, in1=et[:],
                                               op0=OP.mult, op1=OP.max)
                        # b: u = ps*iin; expmap0(u)
                        u = pool.tile([128, 128], F32, tag="u")
                        V.tensor_scalar_mul(u[:], ps[:, 128:256],
                                            scales_sb[:, 2 * T + g:2 * T + g + 1])
                        ss = pool.tile([128, 1], F32, tag="c_ss")
                        S_.activation(junk[:], u[:], AF.Square, accum_out=ss[:])
                        sc = norm_scale_chain(pool, ss, "exp")
                        V.tensor_scalar_mul(nf[:, 128:256], u[:], sc[:, 0:1])
                        # s: l2norm(ps) (inv_in cancels)
                        ss2 = pool.tile([128, 1], F32, tag="c_ss2")
                        S_.activation(junk[:], ps[:, 256:384], AF.Square,
                                      accum_out=ss2[:])
                        sc2 = norm_scale_chain(pool, ss2, "l2")
                        V.tensor_scalar_mul(nf[:, 256:384], ps[:, 256:384],
                                            sc2[:, 0:1])
                        nc.sync.dma_start(feat_dst[g * 128:(g + 1) * 128, :], nf[:])

            # ================= attention + GRU =================
            T_attn = T if "x" in phases else 0
            with (
                tc.tile_pool(name="at", bufs=2) as pool,
                tc.tile_pool(name="atp", bufs=1, space="PSUM") as pp1,
                tc.tile_pool(name="atp2", bufs=2, space="PSUM") as pp2,
            ):
                for t in range(T_attn):
                    cur = pool.tile([128, D], F32, tag="cur")
                    nc.sync.dma_start(cur[:], featB[t * 128:(t + 1) * 128, :])
                    hb = pool.tile([128, W * D], BF, tag="hb")
                    nc.gpsimd.dma_start(hb[:], hist_d[t * 128:(t + 1) * 128, :])

                    curT = []
                    for i in range(3):
                        pt = pp2.tile([128, 128], F32, tag="tp")
                        nc.tensor.transpose(pt[:], cur[:, i * 128:(i + 1) * 128],
                                            idf_sb[:])
                        cT = pool.tile([128, 128], BF, tag=f"cT{i}")
                        V.tensor_copy(cT[:], pt[:])
                        curT.append(cT)

                    ps_q = pp1.tile([128, D], F32, tag="pq")
                    ps_r = pp1.tile([128, D], F32, tag="pr")
                    ps_z = pp1.tile([128, D], F32, tag="pz")
                    ps_n1 = pp1.tile([128, D], F32, tag="pn1")
                    ps_n2 = pp1.tile([128, D], F32, tag="pn2")
                    ps_c = pp1.tile([128, D], F32, tag="pc")

                    # q~ = cur @ Wqk (+bqk)
                    for i in range(3):
                        nc.tensor.matmul(ps_q[:], lhsT=curT[i][:],
                                         rhs=wqk_sb[:, i * D:(i + 1) * D],
                                         start=(i == 0),
                                         stop=(i == 2 and not battn_nz[0]))
                    if battn_nz[0]:
                        nc.tensor.matmul(ps_q[:], lhsT=ones_sb[:],
                                         rhs=battn_sb[0:1, 0:D],
                                         start=False, stop=True)
                    # gi chunks (cur @ Wih)
                    for i in range(3):
                        nc.tensor.matmul(ps_r[:], lhsT=curT[i][:],
                                         rhs=wih_sb[:, i * 3 * D:i * 3 * D + D],
                                         start=(i == 0), stop=False)
                        nc.tensor.matmul(ps_z[:], lhsT=curT[i][:],
                                         rhs=wih_sb[:, i * 3 * D + D:i * 3 * D + 2 * D],
                                         start=(i == 0), stop=False)
                        nc.tensor.matmul(ps_n1[:], lhsT=curT[i][:],
                                         rhs=wih_sb[:, i * 3 * D + 2 * D:(i + 1) * 3 * D],
                                         start=(i == 0),
                                         stop=(i == 2 and not battn_nz[3]))
                    if battn_nz[3]:
                        nc.tensor.matmul(ps_r[:], lhsT=ones_sb[:],
                                         rhs=battn_sb[3:4, 0:D], start=False, stop=False)
                        nc.tensor.matmul(ps_z[:], lhsT=ones_sb[:],
                                         rhs=battn_sb[3:4, D:2 * D], start=False, stop=False)
                        nc.tensor.matmul(ps_n1[:], lhsT=ones_sb[:],
                                         rhs=battn_sb[3:4, 2 * D:3 * D],
                                         start=False, stop=True)

                    qs = pool.tile([128, D], BF, tag="qs")
                    S_.activation(qs[:], ps_q[:], AF.Copy)

                    # scores + softmax over W=5
                    sc_t = pool.tile([128, W], F32, tag="sc")
                    junkb = pool.tile([128, D], BF, tag="junkb")
                    for w in range(W):
                        V.scalar_tensor_tensor(
                            out=junkb[:], in0=qs[:], scalar=1.0,
                            in1=hb[:, w * D:(w + 1) * D],
                            op0=OP.mult, op1=OP.mult,
                            accum_out=sc_t[:, w:w + 1])
                    mx = pool.tile([128, 1], F32, tag="mx")
                    V.reduce_max(mx[:], sc_t[:], axis=mybir.AxisListType.X)
                    nmx = pool.tile([128, 1], F32, tag="nmx")
                    V.tensor_scalar_mul(nmx[:], mx[:], -ID_SCALE)
                    ex = pool.tile([128, W], F32, tag="ex")
                    den = pool.tile([128, 1], F32, tag="den")
                    S_.activation(ex[:], sc_t[:], AF.Exp, bias=nmx[:, 0:1],
                                  scale=ID_SCALE, accum_out=den[:])
                    V.reciprocal(den[:], den[:])
                    at_t = pool.tile([128, W], F32, tag="at")
                    V.tensor_scalar_mul(at_t[:], ex[:], den[:, 0:1])

                    # ctx_pre = sum_w attn_w * hist_w
                    acc = pool.tile([128, D], F32, tag="acc")
                    acc2 = pool.tile([128, D], F32, tag="acc2")
                    V.tensor_scalar_mul(acc[:], hb[:, 0:D], at_t[:, 0:1])
                    for w in range(1, W):
                        a_in, a_out = (acc, acc2) if w % 2 == 1 else (acc2, acc)
                        V.scalar_tensor_tensor(
                            out=a_out[:], in0=hb[:, w * D:(w + 1) * D],
                            scalar=at_t[:, w:w + 1], in1=a_in[:],
                            op0=OP.mult, op1=OP.add)
                    ctx_pre = acc if (W - 1) % 2 == 0 else acc2

                    cpT = []
                    for i in range(3):
                        pt = pp2.tile([128, 128], F32, tag="tp")
                        nc.tensor.transpose(pt[:], ctx_pre[:, i * 128:(i + 1) * 128],
                                            idf_sb[:])
                        cT = pool.tile([128, 128], BF, tag=f"vT{i}")
                        V.tensor_copy(cT[:], pt[:])
                        cpT.append(cT)

                    for i in range(3):
                        nc.tensor.matmul(ps_c[:], lhsT=cpT[i][:],
                                         rhs=wvo_sb[:, i * D:(i + 1) * D],
                                         start=(i == 0),
                                         stop=(i == 2 and not battn_nz[1]))
                        nc.tensor.matmul(ps_r[:], lhsT=cpT[i][:],
                                         rhs=wvog_sb[:, i * 3 * D:i * 3 * D + D],
                                         start=False,
                                         stop=(i == 2 and not battn_nz[2]))
                        nc.tensor.matmul(ps_z[:], lhsT=cpT[i][:],
                                         rhs=wvog_sb[:, i * 3 * D + D:i * 3 * D + 2 * D],
                                         start=False,
                                         stop=(i == 2 and not battn_nz[2]))
                        nc.tensor.matmul(ps_n2[:], lhsT=cpT[i][:],
                                         rhs=wvog_sb[:, i * 3 * D + 2 * D:(i + 1) * 3 * D],
                                         start=(i == 0),
                                         stop=(i == 2 and not battn_nz[2]))
                    if battn_nz[1]:
                        nc.tensor.matmul(ps_c[:], lhsT=ones_sb[:],
                                         rhs=battn_sb[1:2, 0:D], start=False, stop=True)
                    if battn_nz[2]:
                        nc.tensor.matmul(ps_r[:], lhsT=ones_sb[:],
                                         rhs=battn_sb[2:3, 0:D], start=False, stop=True)
                        nc.tensor.matmul(ps_z[:], lhsT=ones_sb[:],
                                         rhs=battn_sb[2:3, D:2 * D], start=False, stop=True)
                        nc.tensor.matmul(ps_n2[:], lhsT=ones_sb[:],
                                         rhs=battn_sb[2:3, 2 * D:3 * D],
                                         start=False, stop=True)

                    ctx = pool.tile([128, D], F32, tag="ctx")
                    S_.activation(ctx[:], ps_c[:], AF.Copy)
                    r_s = pool.tile([128, D], BF, tag="rs")
                    S_.activation(r_s[:], ps_r[:], AF.Sigmoid)
                    z_s = pool.tile([128, D], BF, tag="zs")
                    S_.activation(z_s[:], ps_z[:], AF.Sigmoid)
                    prod = pool.tile([128, D], F32, tag="prod")
                    V.scalar_tensor_tensor(out=prod[:], in0=r_s[:], scalar=1.0,
                                           in1=ps_n2[:], op0=OP.mult, op1=OP.mult)
                    pre = pool.tile([128, D], F32, tag="pre")
                    V.tensor_tensor(out=pre[:], in0=prod[:], in1=ps_n1[:],
                                    op=OP.add)
                    nm = pool.tile([128, D], F32, tag="nm")
                    S_.activation(nm[:], pre[:], AF.Tanh)
                    dd = pool.tile([128, D], F32, tag="dd")
                    V.tensor_tensor(out=dd[:], in0=ctx[:], in1=nm[:],
                                    op=OP.subtract)
                    ot = pool.tile([128, D], F32, tag="ot")
                    V.scalar_tensor_tensor(out=ot[:], in0=z_s[:], scalar=1.0,
                                           in1=dd[:], op0=OP.mult, op1=OP.mult)
                    ot2 = pool.tile([128, D], F32, tag="ot2")
                    V.tensor_tensor(out=ot2[:], in0=ot[:], in1=nm[:], op=OP.add)
                    nc.sync.dma_start(out_d[t * 128:(t + 1) * 128, :], ot2[:])

    nc.compile()
    if os.environ.get("GNN_VERBOSE"):
        n_inst = sum(len(bb.instructions) for f in nc.m.functions for bb in f.blocks)
        print(f"[kernel] instructions: {n_inst}", file=sys.stderr)
    return nc


# ----------------------------------------------------------------- runners

def kernel(**inputs) -> np.ndarray:
    t0 = time.time()
    in_maps, meta = _prep(inputs)
    t1 = time.time()
    nc = _build(meta)
    t2 = time.time()
    res = run_bass_kernel_spmd(nc, in_maps, core_ids=list(range(NCORES)))
    t3 = time.time()
    if os.environ.get("GNN_VERBOSE"):
        print(f"[kernel] prep {t1-t0:.1f}s build+compile {t2-t1:.1f}s "
              f"run {t3-t2:.1f}s", file=sys.stderr)
    _RUN_STATE["nc"] = nc
    _RUN_STATE["in_maps"] = in_maps
    out = np.concatenate([res.results[c]["out"][:OWN] for c in range(NCORES)], 0)
    return out.astype(np.float32)


def bench(iters: int = 8) -> float:
    """Time repeated device executions with pre-staged inputs.

    Returns best wall-clock seconds per execution (includes PJRT dispatch)."""
    import jax
    from jax.sharding import Mesh, PartitionSpec, NamedSharding
    from jax.experimental.shard_map import shard_map
    from concourse import bass2jax
    from concourse.bass2jax import _bass_exec_p, install_neuronx_cc_hook

    nc = _RUN_STATE["nc"]
    in_maps = _RUN_STATE["in_maps"]
    install_neuronx_cc_hook()

    part_name = nc.partition_id_tensor.name if nc.partition_id_tensor else None
    in_names, out_names, out_avals, zero_outs = [], [], [], []
    for alloc in nc.m.functions[0].allocations:
        if not isinstance(alloc, mybir.MemoryLocationSet):
            continue
        name = alloc.memorylocations[0].name
        if alloc.kind == "ExternalInput":
            if name != part_name:
                in_names.append(name)
        elif alloc.kind == "ExternalOutput":
            out_names.append(name)
            shape = tuple(alloc.tensor_shape)
            dtype = mybir.dt.np(alloc.dtype)
            out_avals.append(jax.core.ShapedArray(shape, dtype))
            zero_outs.append(np.zeros(shape, dtype))
    n_params = len(in_names)
    all_names = in_names + out_names
    if part_name is not None:
        all_names = all_names + [part_name]

    def _body(*args):
        operands = list(args)
        if part_name is not None:
            operands.append(bass2jax.partition_id_tensor())
        outs = _bass_exec_p.bind(
            *operands, out_avals=tuple(out_avals), in_names=tuple(all_names),
            out_names=tuple(out_names), lowering_input_output_aliases=(),
            sim_require_finite=True, sim_require_nnan=True, nc=nc)
        return tuple(outs)

    devices = jax.devices()[:NCORES]
    mesh = Mesh(np.asarray(devices), ("core",))
    nin = n_params + len(zero_outs)
    fn = jax.jit(shard_map(_body, mesh=mesh,
                           in_specs=(PartitionSpec("core"),) * nin,
                           out_specs=(PartitionSpec("core"),) * len(out_names),
                           check_rep=False))
    sh = NamedSharding(mesh, PartitionSpec("core"))
    concat_in = [
        jax.device_put(np.concatenate([in_maps[c][k] for c in range(NCORES)], 0), sh)
        for k in in_names
    ] + [
        jax.device_put(np.zeros((NCORES * z.shape[0], *z.shape[1:]), z.dtype), sh)
        for z in zero_outs
    ]
    # warmup (compiles)
    out = fn(*concat_in)
    jax.block_until_ready(out)
    best = float("inf")
    for _ in range(iters):
        t0 = time.perf_counter()
        out = fn(*concat_in)
        jax.block_until_ready(out)
        best = min(best, time.perf_counter() - t0)
    return best


if __name__ == "__main__":
    # lightweight self-run: random inputs of the right shapes
    rng = np.random.default_rng(0)
    demo = {
        "src": rng.integers(0, N, E), "dst": rng.integers(0, N, E),
        "history": rng.standard_normal((W, N, D), dtype=np.float32),
        "node_emb": rng.standard_normal((N, D), dtype=np.float32),
        "e_W": rng.standard_normal((L, SUB, SUB), dtype=np.float32) * SUB ** -0.5,
        "e_b": np.zeros((L, SUB), np.float32),
        "b_W": rng.standard_normal((L, SUB, SUB), dtype=np.float32) * SUB ** -0.5,
        "b_bias": np.zeros((L, SUB), np.float32),
        "s_W": rng.standard_normal((L, SUB, SUB), dtype=np.float32) * SUB ** -0.5,
        "s_bias": np.zeros((L, SUB), np.float32),
        "q_W": rng.standard_normal((D, D), dtype=np.float32) * D ** -0.5,
        "q_b": np.zeros((D,), np.float32),
        "in_w": rng.standard_normal((D, 3 * D), dtype=np.float32) * D ** -0.5,
        "in_b": np.zeros((3 * D,), np.float32),
        "out_w": rng.standard_normal((D, D), dtype=np.float32) * D ** -0.5,
        "out_b": np.zeros((D,), np.float32),
        "gru_wih": rng.standard_normal((D, 3 * D), dtype=np.float32) * D ** -0.5,
        "gru_bih": np.zeros((3 * D,), np.float32),
        "gru_whh": rng.standard_normal((D, 3 * D), dtype=np.float32) * D ** -0.5,
        "gru_bhh": np.zeros((3 * D,), np.float32),
        "curv_b": np.asarray([-1.0], np.float32),
    }
    os.environ["GNN_VERBOSE"] = "1"
    out = kernel(**demo)
    print("out", out.shape, out.dtype, np.abs(out).mean())


# revision 12
# speedup vs baseline: 1.9758x; 1.9758x over previous
"""Trainium2 Bass kernel for nn_APSDG_30124900614677 (gnn_message_passing).

8 NeuronCores, COLLECTIVE-FREE layout (collectives cost ~20ms fixed per
execution in this runtime): every core computes the full graph for layer-1
transform + propagate and layer-2 transform (replicated, in a per-core
node order that puts the core's own nodes first), then computes only its
own 1/8 of layer-2 propagation, history attention and the GRU cell.

Mechanics: node features tiled [128 x 384]; per-layer transforms are PE
matmuls with PE-transposed inputs; propagate = dma_gather of src rows from
a bf16 table + segment-sum via selection-matrix matmuls accumulated in PSUM
(edges pre-sorted by dst on host; selection matrices built on-device from
dst indices with one is_equal per group). Attention/GRU use host-folded
weight products so k/v projections collapse into two matmuls.

Self-contained: hardcodes shapes; imports only numpy/ml_dtypes + the
concourse Bass stack staged at /opt/trn_rl_repo.
"""
import os
import sys
import time

sys.path.insert(0, "/opt/trn_rl_repo")

import numpy as np

from concourse import bass, bacc, mybir, tile
from concourse.bass_utils import run_bass_kernel_spmd

try:
    import ml_dtypes
    BF16 = np.dtype(ml_dtypes.bfloat16)
except Exception:  # pragma: no cover
    import jax.numpy as jnp
    BF16 = np.dtype(jnp.bfloat16)

F32 = mybir.dt.float32
BF = mybir.dt.bfloat16
I16 = mybir.dt.int16
AF = mybir.ActivationFunctionType
OP = mybir.AluOpType

# problem constants
N = 50000
E = 800000
D = 384
SUB = 128
W = 5
L = 2
NCORES = 8
OWN = N // NCORES            # 6250
TB = 49                      # tiles per block
NP = TB * 128                # 6272 padded nodes per block
NG = NCORES * TB             # 392 node tiles total
NPAD = NCORES * NP           # 50176
SPLIT = 32768                # int16 gather-table split row
LEAKY = 0.2
EPS = 1e-5
SEPS = 1e-12
KC = 8                       # dma_gather chunk (64+1 descriptors)
MACRO = 7                    # edge groups per idx/drel table load

_RUN_STATE = {}


def _pad_rows(a, rows):
    out = np.zeros((rows,) + a.shape[1:], a.dtype)
    out[: a.shape[0]] = a
    return out


# ---------------------------------------------------------------- host prep

def _prep(inputs):
    src = np.asarray(inputs["src"]).astype(np.int64)
    dst = np.asarray(inputs["dst"]).astype(np.int64)
    node_emb = np.asarray(inputs["node_emb"], dtype=np.float32)
    history = np.asarray(inputs["history"], dtype=np.float32)

    deg_out = np.maximum(np.bincount(src, minlength=N), 1.0).astype(np.float32)
    deg_in = np.maximum(np.bincount(dst, minlength=N), 1.0).astype(np.float32)
    isqo = deg_out ** -0.5
    isqi = deg_in ** -0.5
    iin = (1.0 / deg_in).astype(np.float32)

    c_abs = float(np.abs(np.asarray(inputs["curv_b"]).reshape(-1)[0]))

    blk = src // OWN
    src_off = src % OWN
    dblk = dst // OWN
    dst_off = dst % OWN

    # per-core local relabeling (rotate blocks so own block is first) and
    # per-(group,class) counts
    percore = []
    cnt = np.zeros((NCORES, NG, 2), np.int64)
    for c in range(NCORES):
        lsrc = ((blk - c) % NCORES) * NP + src_off
        ldst = ((dblk - c) % NCORES) * NP + dst_off
        g = ldst // 128
        cls = (lsrc >= SPLIT).astype(np.int64)
        key = g * 2 + cls
        order = np.argsort(key, kind="stable")
        lsrc, ldst, g, cls, key = (a[order] for a in (lsrc, ldst, g, cls, key))
        cnt[c] += np.bincount(key, minlength=NG * 2).reshape(NG, 2)
        percore.append((lsrc, ldst, g, cls, key))

    KA = np.maximum(1, -(-cnt[:, :, 0].max(0) // 128)).astype(np.int64)
    KB = np.maximum(1, -(-cnt[:, :, 1].max(0) // 128)).astype(np.int64)
    KG = KA + KB
    TILES_TOT = int(KG.sum())
    IDXCOLS = int(8 * KG.sum())

    tile_base = np.zeros((NG, 2), np.int64)
    col_base = np.zeros((NG, 2), np.int64)
    tb = cb = 0
    for g in range(NG):
        tile_base[g, 0] = tb
        tile_base[g, 1] = tb + KA[g]
        col_base[g, 0] = cb
        col_base[g, 1] = cb + 8 * KA[g]
        tb += KG[g]
        cb += 8 * KG[g]

    # ---- fold attention/GRU weights (f64 accumulate) ----
    f64 = np.float64
    q_W = np.asarray(inputs["q_W"], f64)
    q_b = np.asarray(inputs["q_b"], f64)
    in_w = np.asarray(inputs["in_w"], f64)
    in_b = np.asarray(inputs["in_b"], f64)
    out_w = np.asarray(inputs["out_w"], f64)
    out_b = np.asarray(inputs["out_b"], f64)
    gru_wih = np.asarray(inputs["gru_wih"], f64)
    gru_bih = np.asarray(inputs["gru_bih"], f64)
    gru_whh = np.asarray(inputs["gru_whh"], f64)
    gru_bhh = np.asarray(inputs["gru_bhh"], f64)

    Wq = q_W @ in_w[:, :D]
    aq = q_b @ in_w[:, :D] + in_b[:D]
    Wk = in_w[:, D:2 * D]
    Wqk = Wq @ Wk.T
    bqk = aq @ Wk.T
    Wv = in_w[:, 2 * D:]
    bv = in_b[2 * D:]
    Wvo = Wv @ out_w
    bvo = bv @ out_w + out_b
    Wvog = Wvo @ gru_whh
    bvog = bvo @ gru_whh + gru_bhh

    def kmaj(wmat, n):
        return np.ascontiguousarray(
            wmat.reshape(3, 128, n).transpose(1, 0, 2)).astype(BF16)

    wqk_h = kmaj(Wqk, D)
    wvo_h = kmaj(Wvo, D)
    wvog_h = kmaj(Wvog, 3 * D)
    wih_h = kmaj(gru_wih, 3 * D)

    battn = np.zeros((4, 3 * D), np.float64)
    battn[0, :D] = bqk
    battn[1, :D] = bvo
    battn[2] = bvog
    battn[3] = gru_bih
    battn_h = battn.astype(BF16)
    battn_nz = [bool(np.abs(battn[i]).max() > 0) for i in range(4)]

    e_W = np.asarray(inputs["e_W"], np.float32)
    b_W = np.asarray(inputs["b_W"], np.float32)
    s_W = np.asarray(inputs["s_W"], np.float32)
    wsub_h = np.zeros((128, 2 * 3, 128), BF16)
    for l in range(L):
        wsub_h[:, l * 3 + 0] = e_W[l].astype(BF16)
        wsub_h[:, l * 3 + 1] = b_W[l].astype(BF16)
        wsub_h[:, l * 3 + 2] = s_W[l].astype(BF16)

    e_b = np.asarray(inputs["e_b"], np.float32)
    b_bias = np.asarray(inputs["b_bias"], np.float32)
    s_bias = np.asarray(inputs["s_bias"], np.float32)
    brep_h = np.zeros((128, 6, 128), np.float32)
    brep_nz = np.zeros((L, 3), bool)
    for l in range(L):
        for i, b in enumerate((e_b[l], b_bias[l], s_bias[l])):
            brep_h[:, l * 3 + i] = b[None, :]
            brep_nz[l, i] = bool(np.abs(b).max() > 0)

    iota_h = np.tile(np.arange(128, dtype=np.float32)[None, :], (128, 1))
    idf_h = np.eye(128, dtype=np.float32)
    idb_h = np.eye(128, dtype=np.float32).astype(BF16)

    meta = dict(KA=KA, KB=KB, KG=KG, TILES_TOT=TILES_TOT, IDXCOLS=IDXCOLS,
                tile_base=tile_base, col_base=col_base, c_abs=c_abs,
                battn_nz=battn_nz, brep_nz=brep_nz)

    emb_pad = [
        _pad_rows(node_emb[b * OWN:(b + 1) * OWN], NP).astype(BF16)
        for b in range(NCORES)
    ]
    sc_pad = {}
    for nm, arr in (("o", isqo), ("i", isqi), ("n", iin)):
        sc_pad[nm] = [
            np.concatenate([_pad_rows(arr[b * OWN:(b + 1) * OWN, None], NP)[:, 0],
                            ], 0) for b in range(NCORES)
        ]

    in_maps = []
    for c in range(NCORES):
        lsrc, ldst, g, cls, key = percore[c]
        idx16 = np.zeros((16, IDXCOLS), np.int16)
        drel = np.full((128, TILES_TOT), -1.0, np.float32)

        starts = np.zeros(NG * 2, np.int64)
        bc = np.bincount(key, minlength=NG * 2)
        starts[1:] = np.cumsum(bc)[:-1]
        rank = np.arange(len(lsrc)) - starts[key]

        val = np.where(cls == 0, lsrc, lsrc - SPLIT).astype(np.int16)
        col = col_base[g, cls] + rank // 16
        idx16[rank % 16, col] = val

        tt = tile_base[g, cls] + rank // 128
        drel[rank % 128, tt] = (ldst - g * 128).astype(np.float32)

        idx_h = np.tile(idx16, (8, 1))

        scales = np.ones((128, 3 * NG), np.float32)
        for j, nm in enumerate(("o", "i", "n")):
            loc = np.concatenate(
                [sc_pad[nm][(c + lb) % NCORES] for lb in range(NCORES)], 0)
            scales[:, j * NG:(j + 1) * NG] = loc.reshape(NG, 128).T

        feat0 = np.concatenate(
            [emb_pad[(c + lb) % NCORES] for lb in range(NCORES)], 0)

        lo = c * OWN
        hist = np.zeros((NP, W * D), BF16)
        hist[:OWN] = np.ascontiguousarray(
            history[:, lo:lo + OWN, :].transpose(1, 0, 2)
        ).reshape(OWN, W * D).astype(BF16)

        in_maps.append({
            "feat0": feat0,
            "hist": hist,
            "idx": idx_h,
            "drel": drel,
            "scales": scales,
            "wsub": wsub_h.reshape(128, 6 * 128),
            "brep": brep_h.reshape(128, 6 * 128),
            "wqk": wqk_h.reshape(128, 3 * D),
            "wvo": wvo_h.reshape(128, 3 * D),
            "wvog": wvog_h.reshape(128, 9 * D),
            "wih": wih_h.reshape(128, 9 * D),
            "battn": battn_h,
            "iota": iota_h,
            "idf": idf_h,
            "idb": idb_h,
        })
    return in_maps, meta


# ------------------------------------------------------------- device build

def _build(meta):
    KA, KB, KG = meta["KA"], meta["KB"], meta["KG"]
    TILES_TOT, IDXCOLS = meta["TILES_TOT"], meta["IDXCOLS"]
    tile_base, col_base = meta["tile_base"], meta["col_base"]
    c_abs = meta["c_abs"]
    battn_nz = meta["battn_nz"]
    brep_nz = meta["brep_nz"]
    ID_SCALE = float(D) ** -0.5
    phases = os.environ.get("GNN_PHASES", "1ex")  # 1=layer1(full) e=edge2 x=attn

    nc = bacc.Bacc("TRN2", target_bir_lowering=False, debug=False,
                   num_devices=NCORES)

    feat0_d = nc.dram_tensor("feat0", [NPAD, D], BF, kind="ExternalInput")
    hist_d = nc.dram_tensor("hist", [NP, W * D], BF, kind="ExternalInput")
    idx_d = nc.dram_tensor("idx", [128, IDXCOLS], I16, kind="ExternalInput")
    drel_d = nc.dram_tensor("drel", [128, TILES_TOT], F32, kind="ExternalInput")
    scales_d = nc.dram_tensor("scales", [128, 3 * NG], F32, kind="ExternalInput")
    wsub_d = nc.dram_tensor("wsub", [128, 6 * 128], BF, kind="ExternalInput")
    brep_d = nc.dram_tensor("brep", [128, 6 * 128], F32, kind="ExternalInput")
    wqk_d = nc.dram_tensor("wqk", [128, 3 * D], BF, kind="ExternalInput")
    wvo_d = nc.dram_tensor("wvo", [128, 3 * D], BF, kind="ExternalInput")
    wvog_d = nc.dram_tensor("wvog", [128, 9 * D], BF, kind="ExternalInput")
    wih_d = nc.dram_tensor("wih", [128, 9 * D], BF, kind="ExternalInput")
    battn_d = nc.dram_tensor("battn", [4, 3 * D], BF, kind="ExternalInput")
    iota_d = nc.dram_tensor("iota", [128, 128], F32, kind="ExternalInput")
    idf_d = nc.dram_tensor("idf", [128, 128], F32, kind="ExternalInput")
    idb_d = nc.dram_tensor("idb", [128, 128], BF, kind="ExternalInput")
    out_d = nc.dram_tensor("out", [NP, D], F32, kind="ExternalOutput")

    t1 = nc.dram_tensor("t1", [NPAD, D], BF, kind="Internal")
    t2 = nc.dram_tensor("t2", [NPAD, D], BF, kind="Internal")
    feat1 = nc.dram_tensor("feat1", [NPAD, D], BF, kind="Internal")
    feat2 = nc.dram_tensor("feat2", [NP, D], F32, kind="Internal")

    with tile.TileContext(nc) as tc:
        with tc.tile_pool(name="const", bufs=1) as cpool:
            scales_sb = cpool.tile([128, 3 * NG], F32)
            wsub_sb = cpool.tile([128, 6 * 128], BF)
            brep_sb = cpool.tile([128, 6 * 128], F32)
            wqk_sb = cpool.tile([128, 3 * D], BF)
            wvo_sb = cpool.tile([128, 3 * D], BF)
            wvog_sb = cpool.tile([128, 9 * D], BF)
            wih_sb = cpool.tile([128, 9 * D], BF)
            battn_sb = cpool.tile([4, 3 * D], BF)
            iota_sb = cpool.tile([128, 128], F32)
            idf_sb = cpool.tile([128, 128], F32)
            idb_sb = cpool.tile([128, 128], BF)
            ones_sb = cpool.tile([1, 128], BF)
            for sb, dr in ((scales_sb, scales_d), (wsub_sb, wsub_d),
                           (brep_sb, brep_d), (wqk_sb, wqk_d),
                           (wvo_sb, wvo_d), (wvog_sb, wvog_d),
                           (wih_sb, wih_d), (battn_sb, battn_d),
                           (iota_sb, iota_d), (idf_sb, idf_d),
                           (idb_sb, idb_d)):
                nc.sync.dma_start(sb[:], dr[:])
            nc.gpsimd.memset(ones_sb[:], 1.0)

            V = nc.vector
            S_ = nc.scalar

            def norm_scale_chain(pool, ss, kind):
                """[128,1] f32 chains on ss = sum(x^2).
                'log': artanh(min(sn,1-eps))/max(sn,eps); 'exp': tanh(sn)/max(sn,eps);
                'l2': 1/max(sqrt(ss),1e-12); sn = sqrt(c_abs*ss)."""
                sn = pool.tile([128, 1], F32, tag="c_sn")
                S_.activation(sn[:], ss[:], AF.Sqrt,
                              scale=c_abs if kind != "l2" else 1.0)
                m = pool.tile([128, 1], F32, tag="c_m")
                V.tensor_scalar_max(m[:], sn[:], SEPS if kind == "l2" else EPS)
                V.reciprocal(m[:], m[:])
                if kind == "l2":
                    return m
                if kind == "exp":
                    th = pool.tile([128, 1], F32, tag="c_th")
                    S_.activation(th[:], sn[:], AF.Tanh)
                    sc = pool.tile([128, 1], F32, tag="c_sc")
                    V.tensor_tensor(out=sc[:], in0=th[:], in1=m[:], op=OP.mult)
                    return sc
                x = pool.tile([128, 1], F32, tag="c_x")
                V.tensor_scalar_min(x[:], sn[:], 1.0 - EPS)
                a1 = pool.tile([128, 1], F32, tag="c_a1")
                V.tensor_scalar_add(a1[:], x[:], 1.0)
                a2 = pool.tile([128, 1], F32, tag="c_a2")
                V.tensor_scalar(a2[:], x[:], -1.0, 1.0, op0=OP.mult, op1=OP.add)
                V.reciprocal(a2[:], a2[:])
                y = pool.tile([128, 1], F32, tag="c_y")
                V.tensor_tensor(out=y[:], in0=a1[:], in1=a2[:], op=OP.mult)
                ln = pool.tile([128, 1], F32, tag="c_ln")
                S_.activation(ln[:], y[:], AF.Ln)
                sc = pool.tile([128, 1], F32, tag="c_sc")
                V.scalar_tensor_tensor(out=sc[:], in0=ln[:], scalar=0.5,
                                       in1=m[:], op0=OP.mult, op1=OP.mult)
                return sc

            def transform(l, feat_src, t_dst, ntiles):
                with (
                    tc.tile_pool(name=f"tf{l}", bufs=3) as pool,
                    tc.tile_pool(name=f"tfp{l}", bufs=2, space="PSUM") as pp,
                ):
                    for t in range(ntiles):
                        ft = pool.tile([128, D], BF, tag="ft")
                        nc.sync.dma_start(ft[:], feat_src[t * 128:(t + 1) * 128, :])
                        tsb = pool.tile([128, D], BF, tag="tsb")
                        junk = pool.tile([128, 128], BF, tag="junk")
                        for i, kind in enumerate(("e", "b", "s")):
                            sl = slice(i * 128, (i + 1) * 128)
                            if kind == "e":
                                xin = ft[:, sl]
                            else:
                                ss = pool.tile([128, 1], F32, tag="c_ss")
                                S_.activation(junk[:], ft[:, sl], AF.Square,
                                              accum_out=ss[:])
                                sc = norm_scale_chain(
                                    pool, ss, "log" if kind == "b" else "l2")
                                xs = pool.tile([128, 128], BF, tag="xs")
                                V.tensor_scalar_mul(xs[:], ft[:, sl], sc[:, 0:1])
                                xin = xs[:]
                            pt = pp.tile([128, 128], BF, tag="tp")
                            nc.tensor.transpose(pt[:], xin, idb_sb[:])
                            xT = pool.tile([128, 128], BF, tag="xT")
                            V.tensor_copy(xT[:], pt[:])
                            pm = pp.tile([128, 128], F32, tag="mm")
                            nc.tensor.matmul(
                                pm[:], lhsT=xT[:],
                                rhs=wsub_sb[:, (l * 3 + i) * 128:(l * 3 + i + 1) * 128],
                                start=True, stop=True)
                            if kind == "e":
                                V.tensor_scalar_mul(tsb[:, sl], pm[:],
                                                    scales_sb[:, t:t + 1])
                            elif brep_nz[l][i]:
                                V.scalar_tensor_tensor(
                                    out=tsb[:, sl], in0=pm[:], scalar=1.0,
                                    in1=brep_sb[:, (l * 3 + i) * 128:(l * 3 + i + 1) * 128],
                                    op0=OP.mult, op1=OP.add)
                            else:
                                V.tensor_copy(tsb[:, sl], pm[:])
                        nc.sync.dma_start(t_dst[t * 128:(t + 1) * 128, :], tsb[:])

            def edge(l, t_src, feat_dst, ngroups, out_f32):
                with (
                    tc.tile_pool(name=f"ed{l}", bufs=2) as pool,
                    tc.tile_pool(name=f"edp{l}", bufs=2, space="PSUM") as pp,
                ):
                    KGmax = int(KG[:ngroups].max())
                    mac = [(m, min(m + MACRO, ngroups))
                           for m in range(0, ngroups, MACRO)]
                    max_cols = 0
                    max_tls = 0
                    for a, b in mac:
                        ce = int(col_base[b, 0]) if b < NG else IDXCOLS
                        te = int(tile_base[b, 0]) if b < NG else TILES_TOT
                        max_cols = max(max_cols, ce - int(col_base[a, 0]))
                        max_tls = max(max_tls, te - int(tile_base[a, 0]))
                    for a, b in mac:
                        c0 = int(col_base[a, 0])
                        t0 = int(tile_base[a, 0])
                        ce = int(col_base[b, 0]) if b < NG else IDXCOLS
                        te = int(tile_base[b, 0]) if b < NG else TILES_TOT
                        idxm = pool.tile([128, max_cols], I16, tag="idxm")
                        nc.sync.dma_start(idxm[:, :ce - c0], idx_d[:, c0:ce])
                        drm = pool.tile([128, max_tls], F32, tag="drm")
                        nc.sync.dma_start(drm[:, :te - t0], drel_d[:, t0:te])
                        for g in range(a, b):
                            ka, kb, kg = int(KA[g]), int(KB[g]), int(KG[g])
                            ca = int(col_base[g, 0]) - c0
                            cbb = int(col_base[g, 1]) - c0
                            tb_ = int(tile_base[g, 0]) - t0
                            msg = pool.tile([128, KGmax, D], BF, tag="msg")

                            def gather(table_ap, kcnt, colofs, chunk0):
                                for q0 in range(0, kcnt, KC):
                                    kc = min(KC, kcnt - q0)
                                    nc.gpsimd.dma_gather(
                                        out_ap=msg[:, chunk0 + q0:chunk0 + q0 + kc, :],
                                        in_ap=table_ap,
                                        idxs_ap=idxm[:, colofs + 8 * q0:
                                                     colofs + 8 * (q0 + kc)],
                                        num_idxs=128 * kc, num_idxs_reg=128 * kc,
                                        elem_size=D)

                            gather(t_src[0:SPLIT, :], ka, ca, 0)
                            gather(t_src[SPLIT:NPAD, :], kb, cbb, ka)

                            Sall = pool.tile([128, KGmax * 128], BF, tag="S")
                            V.tensor_tensor(
                                out=Sall[:].rearrange("p (k j) -> p k j", j=128)[:, :kg, :],
                                in0=drm[:, tb_:tb_ + kg].to_broadcast([128, kg, 128]),
                                in1=iota_sb[:, None, :].to_broadcast([128, kg, 128]),
                                op=OP.is_equal)
                            ps = pp.tile([128, D], F32, tag="eps")
                            for k in range(kg):
                                nc.tensor.matmul(ps[:],
                                                 lhsT=Sall[:, k * 128:(k + 1) * 128],
                                                 rhs=msg[:, k, :],
                                                 start=(k == 0), stop=(k == kg - 1))
                            nf = pool.tile([128, D], F32 if out_f32 else BF,
                                           tag=f"nf{l}")
                            junk = pool.tile([128, 128], BF, tag="junk")
                            et = pool.tile([128, 128], F32, tag="et")
                            if brep_nz[l][0]:
                                V.scalar_tensor_tensor(
                                    out=et[:], in0=ps[:, 0:128],
                                    scalar=scales_sb[:, NG + g:NG + g + 1],
                                    in1=brep_sb[:, (l * 3) * 128:(l * 3 + 1) * 128],
                                    op0=OP.mult, op1=OP.add)
                            else:
                                V.tensor_scalar_mul(et[:], ps[:, 0:128],
                                                    scales_sb[:, NG + g:NG + g + 1])
                            V.scalar_tensor_tensor(out=nf[:, 0:128], in0=et[:],
                                                   scalar=LEAKY, in1=et[:],
                                                   op0=OP.mult, op1=OP.max)
                            u = pool.tile([128, 128], F32, tag="u")
                            V.tensor_scalar_mul(u[:], ps[:, 128:256],
                                                scales_sb[:, 2 * NG + g:2 * NG + g + 1])
                            ss = pool.tile([128, 1], F32, tag="c_ss")
                            S_.activation(junk[:], u[:], AF.Square, accum_out=ss[:])
                            sc = norm_scale_chain(pool, ss, "exp")
                            V.tensor_scalar_mul(nf[:, 128:256], u[:], sc[:, 0:1])
                            ss2 = pool.tile([128, 1], F32, tag="c_ss2")
                            S_.activation(junk[:], ps[:, 256:384], AF.Square,
                                          accum_out=ss2[:])
                            sc2 = norm_scale_chain(pool, ss2, "l2")
                            V.tensor_scalar_mul(nf[:, 256:384], ps[:, 256:384],
                                                sc2[:, 0:1])
                            nc.sync.dma_start(
                                feat_dst[g * 128:(g + 1) * 128, :], nf[:])

            # ================= layers =================
            if "1" in phases:
                transform(0, feat0_d, t1, NG)
                edge(0, t1, feat1, NG, out_f32=False)
                transform(1, feat1, t2, NG)
            if "e" in phases:
                edge(1, t2, feat2, TB, out_f32=True)

            # ================= attention + GRU =================
            T_attn = TB if "x" in phases else 0
            with (
                tc.tile_pool(name="at", bufs=2) as pool,
                tc.tile_pool(name="atp", bufs=1, space="PSUM") as pp1,
                tc.tile_pool(name="atp2", bufs=2, space="PSUM") as pp2,
            ):
                for t in range(T_attn):
                    cur = pool.tile([128, D], F32, tag="cur")
                    nc.sync.dma_start(cur[:], feat2[t * 128:(t + 1) * 128, :])
                    hb = pool.tile([128, W * D], BF, tag="hb")
                    nc.sync.dma_start(hb[:], hist_d[t * 128:(t + 1) * 128, :])

                    curT = []
                    for i in range(3):
                        pt = pp2.tile([128, 128], F32, tag="tp")
                        nc.tensor.transpose(pt[:], cur[:, i * 128:(i + 1) * 128],
                                            idf_sb[:])
                        cT = pool.tile([128, 128], BF, tag=f"cT{i}")
                        V.tensor_copy(cT[:], pt[:])
                        curT.append(cT)

                    ps_q = pp1.tile([128, D], F32, tag="pq")
                    ps_r = pp1.tile([128, D], F32, tag="pr")
                    ps_z = pp1.tile([128, D], F32, tag="pz")
                    ps_n1 = pp1.tile([128, D], F32, tag="pn1")
                    ps_n2 = pp1.tile([128, D], F32, tag="pn2")
                    ps_c = pp1.tile([128, D], F32, tag="pc")

                    for i in range(3):
                        nc.tensor.matmul(ps_q[:], lhsT=curT[i][:],
                                         rhs=wqk_sb[:, i * D:(i + 1) * D],
                                         start=(i == 0),
                                         stop=(i == 2 and not battn_nz[0]))
                    if battn_nz[0]:
                        nc.tensor.matmul(ps_q[:], lhsT=ones_sb[:],
                                         rhs=battn_sb[0:1, 0:D],
                                         start=False, stop=True)
                    for i in range(3):
                        nc.tensor.matmul(ps_r[:], lhsT=curT[i][:],
                                         rhs=wih_sb[:, i * 3 * D:i * 3 * D + D],
                                         start=(i == 0), stop=False)
                        nc.tensor.matmul(ps_z[:], lhsT=curT[i][:],
                                         rhs=wih_sb[:, i * 3 * D + D:i * 3 * D + 2 * D],
                                         start=(i == 0), stop=False)
                        nc.tensor.matmul(ps_n1[:], lhsT=curT[i][:],
                                         rhs=wih_sb[:, i * 3 * D + 2 * D:(i + 1) * 3 * D],
                                         start=(i == 0),
                                         stop=(i == 2 and not battn_nz[3]))
                    if battn_nz[3]:
                        nc.tensor.matmul(ps_r[:], lhsT=ones_sb[:],
                                         rhs=battn_sb[3:4, 0:D], start=False,
                                         stop=False)
                        nc.tensor.matmul(ps_z[:], lhsT=ones_sb[:],
                                         rhs=battn_sb[3:4, D:2 * D], start=False,
                                         stop=False)
                        nc.tensor.matmul(ps_n1[:], lhsT=ones_sb[:],
                                         rhs=battn_sb[3:4, 2 * D:3 * D],
                                         start=False, stop=True)

                    qs = pool.tile([128, D], BF, tag="qs")
                    S_.activation(qs[:], ps_q[:], AF.Copy)

                    sc_t = pool.tile([128, W], F32, tag="sc")
                    junkb = pool.tile([128, D], BF, tag="junkb")
                    for w in range(W):
                        V.scalar_tensor_tensor(
                            out=junkb[:], in0=qs[:], scalar=1.0,
                            in1=hb[:, w * D:(w + 1) * D],
                            op0=OP.mult, op1=OP.mult,
                            accum_out=sc_t[:, w:w + 1])
                    mx = pool.tile([128, 1], F32, tag="mx")
                    V.reduce_max(mx[:], sc_t[:], axis=mybir.AxisListType.X)
                    nmx = pool.tile([128, 1], F32, tag="nmx")
                    V.tensor_scalar_mul(nmx[:], mx[:], -ID_SCALE)
                    ex = pool.tile([128, W], F32, tag="ex")
                    den = pool.tile([128, 1], F32, tag="den")
                    S_.activation(ex[:], sc_t[:], AF.Exp, bias=nmx[:, 0:1],
                                  scale=ID_SCALE, accum_out=den[:])
                    V.reciprocal(den[:], den[:])
                    at_t = pool.tile([128, W], F32, tag="at")
                    V.tensor_scalar_mul(at_t[:], ex[:], den[:, 0:1])

                    acc = pool.tile([128, D], F32, tag="acc")
                    acc2 = pool.tile([128, D], F32, tag="acc2")
                    V.tensor_scalar_mul(acc[:], hb[:, 0:D], at_t[:, 0:1])
                    for w in range(1, W):
                        a_in, a_out = (acc, acc2) if w % 2 == 1 else (acc2, acc)
                        V.scalar_tensor_tensor(
                            out=a_out[:], in0=hb[:, w * D:(w + 1) * D],
                            scalar=at_t[:, w:w + 1], in1=a_in[:],
                            op0=OP.mult, op1=OP.add)
                    ctx_pre = acc if (W - 1) % 2 == 0 else acc2

                    cpT = []
                    for i in range(3):
                        pt = pp2.tile([128, 128], F32, tag="tp")
                        nc.tensor.transpose(pt[:], ctx_pre[:, i * 128:(i + 1) * 128],
                                            idf_sb[:])
                        cT = pool.tile([128, 128], BF, tag=f"vT{i}")
                        V.tensor_copy(cT[:], pt[:])
                        cpT.append(cT)

                    for i in range(3):
                        nc.tensor.matmul(ps_c[:], lhsT=cpT[i][:],
                                         rhs=wvo_sb[:, i * D:(i + 1) * D],
                                         start=(i == 0),
                                         stop=(i == 2 and not battn_nz[1]))
                        nc.tensor.matmul(ps_r[:], lhsT=cpT[i][:],
                                         rhs=wvog_sb[:, i * 3 * D:i * 3 * D + D],
                                         start=False,
                                         stop=(i == 2 and not battn_nz[2]))
                        nc.tensor.matmul(ps_z[:], lhsT=cpT[i][:],
                                         rhs=wvog_sb[:, i * 3 * D + D:i * 3 * D + 2 * D],
                                         start=False,
                                         stop=(i == 2 and not battn_nz[2]))
                        nc.tensor.matmul(ps_n2[:], lhsT=cpT[i][:],
                                         rhs=wvog_sb[:, i * 3 * D + 2 * D:(i + 1) * 3 * D],
                                         start=(i == 0),
                                         stop=(i == 2 and not battn_nz[2]))
                    if battn_nz[1]:
                        nc.tensor.matmul(ps_c[:], lhsT=ones_sb[:],
                                         rhs=battn_sb[1:2, 0:D], start=False,
                                         stop=True)
                    if battn_nz[2]:
                        nc.tensor.matmul(ps_r[:], lhsT=ones_sb[:],
                                         rhs=battn_sb[2:3, 0:D], start=False,
                                         stop=True)
                        nc.tensor.matmul(ps_z[:], lhsT=ones_sb[:],
                                         rhs=battn_sb[2:3, D:2 * D], start=False,
                                         stop=True)
                        nc.tensor.matmul(ps_n2[:], lhsT=ones_sb[:],
                                         rhs=battn_sb[2:3, 2 * D:3 * D],
                                         start=False, stop=True)

                    ctx = pool.tile([128, D], F32, tag="ctx")
                    S_.activation(ctx[:], ps_c[:], AF.Copy)
                    r_s = pool.tile([128, D], BF, tag="rs")
                    S_.activation(r_s[:], ps_r[:], AF.Sigmoid)
                    z_s = pool.tile([128, D], BF, tag="zs")
                    S_.activation(z_s[:], ps_z[:], AF.Sigmoid)
                    prod = pool.tile([128, D], F32, tag="prod")
                    V.scalar_tensor_tensor(out=prod[:], in0=r_s[:], scalar=1.0,
                                           in1=ps_n2[:], op0=OP.mult, op1=OP.mult)
                    pre = pool.tile([128, D], F32, tag="pre")
                    V.tensor_tensor(out=pre[:], in0=prod[:], in1=ps_n1[:],
                                    op=OP.add)
                    nm = pool.tile([128, D], F32, tag="nm")
                    S_.activation(nm[:], pre[:], AF.Tanh)
                    dd = pool.tile([128, D], F32, tag="dd")
                    V.tensor_tensor(out=dd[:], in0=ctx[:], in1=nm[:],
                                    op=OP.subtract)
                    ot = pool.tile([128, D], F32, tag="ot")
                    V.scalar_tensor_tensor(out=ot[:], in0=z_s[:], scalar=1.0,
                                           in1=dd[:], op0=OP.mult, op1=OP.mult)
                    ot2 = pool.tile([128, D], F32, tag="ot2")
                    V.tensor_tensor(out=ot2[:], in0=ot[:], in1=nm[:], op=OP.add)
                    nc.sync.dma_start(out_d[t * 128:(t + 1) * 128, :], ot2[:])

    nc.compile()
    if os.environ.get("GNN_VERBOSE"):
        n_inst = sum(len(bb.instructions) for f in nc.m.functions for bb in f.blocks)
        print(f"[kernel] instructions: {n_inst}", file=sys.stderr)
    return nc


# ----------------------------------------------------------------- runners

def kernel(**inputs) -> np.ndarray:
    t0 = time.time()
    in_maps, meta = _prep(inputs)
    t1 = time.time()
    nc = _build(meta)
    t2 = time.time()
    res = run_bass_kernel_spmd(nc, in_maps, core_ids=list(range(NCORES)))
    t3 = time.time()
    if os.environ.get("GNN_VERBOSE"):
        print(f"[kernel] prep {t1-t0:.1f}s build+compile {t2-t1:.1f}s "
              f"run {t3-t2:.1f}s", file=sys.stderr)
    _RUN_STATE["nc"] = nc
    _RUN_STATE["in_maps"] = in_maps
    out = np.concatenate([res.results[c]["out"][:OWN] for c in range(NCORES)], 0)
    return out.astype(np.float32)


def bench(iters: int = 8) -> float:
    """Best wall-clock seconds per device execution with pre-staged inputs
    (includes the runtime's fixed dispatch overhead)."""
    import jax
    from jax.sharding import Mesh, PartitionSpec, NamedSharding
    from jax.experimental.shard_map import shard_map
    from concourse import bass2jax
    from concourse.bass2jax import _bass_exec_p, install_neuronx_cc_hook

    nc = _RUN_STATE["nc"]
    in_maps = _RUN_STATE["in_maps"]
    install_neuronx_cc_hook()

    part_name = nc.partition_id_tensor.name if nc.partition_id_tensor else None
    in_names, out_names, out_avals, zero_outs = [], [], [], []
    for alloc in nc.m.functions[0].allocations:
        if not isinstance(alloc, mybir.MemoryLocationSet):
            continue
        name = alloc.memorylocations[0].name
        if alloc.kind == "ExternalInput":
            if name != part_name:
                in_names.append(name)
        elif alloc.kind == "ExternalOutput":
            out_names.append(name)
            shape = tuple(alloc.tensor_shape)
            dtype = mybir.dt.np(alloc.dtype)
            out_avals.append(jax.core.ShapedArray(shape, dtype))
            zero_outs.append(np.zeros(shape, dtype))
    n_params = len(in_names)
    all_names = in_names + out_names
    if part_name is not None:
        all_names = all_names + [part_name]

    def _body(*args):
        operands = list(args)
        if part_name is not None:
            operands.append(bass2jax.partition_id_tensor())
        outs = _bass_exec_p.bind(
            *operands, out_avals=tuple(out_avals), in_names=tuple(all_names),
            out_names=tuple(out_names), lowering_input_output_aliases=(),
            sim_require_finite=True, sim_require_nnan=True, nc=nc)
        return tuple(outs)

    devices = jax.devices()[:NCORES]
    mesh = Mesh(np.asarray(devices), ("core",))
    nin = n_params + len(zero_outs)
    fn = jax.jit(shard_map(_body, mesh=mesh,
                           in_specs=(PartitionSpec("core"),) * nin,
                           out_specs=(PartitionSpec("core"),) * len(out_names),
                           check_rep=False))
    sh = NamedSharding(mesh, PartitionSpec("core"))
    concat_in = [
        jax.device_put(np.concatenate([in_maps[c][k] for c in range(NCORES)], 0), sh)
        for k in in_names
    ] + [
        jax.device_put(np.zeros((NCORES * z.shape[0], *z.shape[1:]), z.dtype), sh)
        for z in zero_outs
    ]
    out = fn(*concat_in)
    jax.block_until_ready(out)
    best = float("inf")
    for _ in range(iters):
        t0 = time.perf_counter()
        out = fn(*concat_in)
        jax.block_until_ready(out)
        best = min(best, time.perf_counter() - t0)
    return best
